# revision 21
# baseline (speedup 1.0000x reference)
"""Trainium2 Bass kernel for nn_DiagLRConv (diag-embedded 5x5 conv, pad=2).

Math: out[n,o,h,w] = sum_{i,k} filter_w[o,i,k] * x[n,i,h+k-2,w+k-2]
(a diag_embed'ed 5x5 kernel is 5 diagonal shifts mixed through 16x16 channel
matrices).

Mapping (per NeuronCore, 2 images each, 8 cores data-parallel over batch):
  - x cast to fp16 and zero-padded on host into a flat [2,16,(H+5)*517]
    layout (517 = 2 + 512 + 3 pad columns).  fp16 rounding of x/w is the
    only approximation (~3e-4 rel l2, threshold 2e-2).
  - x is loaded ONCE (no shifted duplicate reads): each 128-row slab is
    4 row-bands of 32 output rows; band i occupies partitions 32i..32i+32
    holding [img0 16ch; img1 16ch] x 37 padded rows x 517 cols, loaded as
    one flat contiguous 38 KB/partition DMA run per (band, image).
  - Diagonal tap k of output row t reads the flat buffer at offset
    (row_in_buf)*517 + k -- no pre-shifted copies needed.
  - Matmul: 16 concurrent 32x32 tiles via tile_position=(32i,32j):
    row-band i = x data band, col-band j = output row t=4s+j.  Stationary
    [K=32,N=32] is block-diagonal: cols 0:16 = img0 out channels, cols
    16:32 = img1, so each tile computes both images at once.  5 tap-rounds
    accumulate into PSUM bank i (4 banks/step, 8 banks double-buffered);
    concurrent tiles on one column strip always target different banks.
  - PSUM -> SBUF evacuation with fp32->fp16 cast as one 2-bank (1024-col)
    copy per engine per step -- ScalarE takes bands 0,1, VectorE bands
    2,3 -- amortizing the TRN2 SBUF-op inter-instruction bubble; one
    512 KB output DMA per step on the GpSimd SWDGE path in a
    kernel-native layout; host reassembles.
"""

import numpy as np

F16 = np.float16

_COMPILED = {}

ROWS_PER_BAND = 32            # output rows per row-band per slab
BANDS = 4
SLAB = ROWS_PER_BAND * BANDS  # 128 output rows per slab
RB = ROWS_PER_BAND + 5        # 37 buffer rows per band
WPAD = 517                    # padded row length (2 + 512 + 3)
L = RB * WPAD                 # flat fp16 elems per partition per slab
STEPS = ROWS_PER_BAND // 4    # 8 steps per slab (4 rows per step per band)


def _trace_nc(H):
    import concourse.mybir as mybir
    import concourse.tile as tile
    from concourse import bacc

    F32 = mybir.dt.float32
    FP16 = mybir.dt.float16

    assert H % SLAB == 0
    G = H // SLAB

    nc = bacc.Bacc(None, target_bir_lowering=False, debug=False)
    # banded input layout, host-materialized: xp[g, 32i+16m+c, r*517+w] =
    # xpad[m, c, 128g+32i+r, w] -- so each input DMA spans all 128
    # partitions with ~5 KB/partition descriptors (DMA engines pipeline
    # across partitions only at descriptor granularity).
    xp = nc.declare_dram_parameter("xp", [G, 128, L], FP16, isOutput=False)
    wd = nc.declare_dram_parameter("wd", [128, 5, 32], FP16, isOutput=False)
    # kernel-native output layout; host reassembles:
    # y[g, s, 32j+16m+o, i, w] = out[m, o, 128g+32i+4s+j, w]
    y = nc.declare_dram_parameter("y", [G, STEPS, 128, 4, 512], FP16, isOutput=True)

    with tile.TileContext(nc) as tc:
        with (
            tc.tile_pool(name="const", bufs=1) as const,
            tc.tile_pool(name="xpool", bufs=4) as xpool,
            tc.tile_pool(name="psum", bufs=4, space="PSUM") as psum,
            tc.tile_pool(name="stpool", bufs=4) as stpool,
        ):
            wt = const.tile([128, 5, 32], FP16)
            nc.sync.dma_start(out=wt[:], in_=wd[:])

            CHUNK = 3 * WPAD  # ~3.1 KB/partition per descriptor; finer
            # chunks shrink the just-in-time arrival stalls when compute
            # catches up to the contended input stream mid-slab
            for g in range(G):
                xq = xpool.tile([128, L], FP16, tag="xq", name=f"xq{g}")
                for c0 in range(0, L, CHUNK):
                    c1 = min(L, c0 + CHUNK)
                    nc.sync.dma_start(out=xq[:, c0:c1], in_=xp[g, :, c0:c1])
                for s in range(STEPS):
                    # two 2-bank PSUM tiles per step: band i -> pair i//2,
                    # bank i%2; evac is one 1024-col copy per engine
                    pss = [
                        psum.tile([128, 2, 512], F32, tag="ps", name=f"ps{g}_{s}_{p}")
                        for p in range(2)
                    ]
                    st = stpool.tile([128, 4, 512], FP16, tag="st", name=f"st{g}_{s}")
                    for k in range(5):
                        for i in range(BANDS):
                            for j in range(4):
                                off = (4 * s + j + k) * WPAD + k
                                nc.tensor.matmul(
                                    pss[i // 2][32 * j : 32 * j + 32, i % 2, :],
                                    wt[32 * i : 32 * i + 32, k, :],
                                    xq[32 * i : 32 * i + 32, off : off + 512],
                                    start=(k == 0),
                                    stop=(k == 4),
                                    tile_position=(32 * i, 32 * j),
                                    skip_group_check=True,
                                )
                    nc.scalar.copy(st[:, 0:2, :], pss[0][:])
                    nc.vector.tensor_copy(st[:, 2:4, :], pss[1][:])
                    # SWDGE (gpsimd) output path: separate descriptor
                    # queues from the sync-ring input stream, so output
                    # transfers interleave with input at packet granularity
                    nc.gpsimd.dma_start(out=y[g, s], in_=st[:])
    nc.compile()
    return nc


def _get_nc(H, **kw):
    key = (H, tuple(sorted(kw.items())))
    if key not in _COMPILED:
        _COMPILED[key] = _trace_nc(H, **kw)
    return _COMPILED[key]


def _prep_inputs(x, filter_w, H):
    """x: [N,16,H,512] fp32, filter_w: [16,16,5] fp32 -> per-core in_maps."""
    N = x.shape[0]
    n_cores = N // 2
    x16 = x.astype(F16)

    wT = np.transpose(filter_w.astype(F16), (1, 2, 0))  # [i, k, o]
    wd = np.zeros((128, 5, 32), dtype=F16)
    for b in range(BANDS):
        wd[32 * b : 32 * b + 16, :, 0:16] = wT
        wd[32 * b + 16 : 32 * b + 32, :, 16:32] = wT

    G = H // SLAB
    row_starts = (
        np.arange(G)[:, None] * SLAB + np.arange(BANDS)[None, :] * ROWS_PER_BAND
    )  # [G, BANDS]
    in_maps = []
    for cid in range(n_cores):
        xpf = np.zeros((2, 16, H + 5, WPAD), dtype=F16)
        xpf[:, :, 2 : H + 2, 2:514] = x16[2 * cid : 2 * cid + 2]
        # banded layout [G, 128, L]: partition 32i+16m+c holds band i's
        # RB padded rows (with halo duplicated across bands)
        xb = xpf[:, :, row_starts[:, :, None] + np.arange(RB)]  # [2,16,G,4,RB,517]
        xb = np.transpose(xb, (2, 3, 0, 1, 4, 5)).reshape(G, 128, L)
        in_maps.append({"xp": np.ascontiguousarray(xb), "wd": wd})
    return in_maps


def _reassemble(yk, H):
    # yk [G, STEPS, 128, 4, 512]; p = 32j + 16m + o; row = 128g + 32i + 4s + j
    G = H // SLAB
    z = yk.reshape(G, STEPS, 4, 2, 16, 4, 512)      # g, s, j, m, o, i, w
    z = np.transpose(z, (3, 4, 0, 5, 1, 2, 6))      # m, o, g, i, s, j, w
    return z.reshape(2, 16, H, 512).astype(np.float32)


def kernel(x, filter_w):
    from concourse.bass_utils import run_bass_kernel_spmd

    x = np.asarray(x)
    filter_w = np.asarray(filter_w)
    N, C, H, W = x.shape
    assert (C, W) == (16, 512) and N % 2 == 0

    nc = _get_nc(H)
    in_maps = _prep_inputs(x, filter_w, H)
    n_cores = len(in_maps)
    res = run_bass_kernel_spmd(nc, in_maps, list(range(n_cores)))
    out = np.empty((N, 16, H, 512), dtype=np.float32)
    for cid in range(n_cores):
        out[2 * cid : 2 * cid + 2] = _reassemble(res.results[cid]["y"], H)
    return out


if __name__ == "__main__":
    import sys

    H = int(sys.argv[1]) if len(sys.argv) > 1 else 128
    rng = np.random.default_rng(0)
    x = rng.standard_normal((16, 16, H, 512)).astype(np.float32)
    fw = (rng.standard_normal((16, 16, 5)) * 0.1).astype(np.float32)
    out = kernel(x, fw)

    xpad = np.zeros((16, 16, H + 4, 516), dtype=np.float64)
    xpad[:, :, 2 : H + 2, 2:514] = x
    ref = np.zeros_like(out, dtype=np.float64)
    for k in range(5):
        sh = xpad[:, :, k : k + H, k : k + 512]
        ref += np.einsum("oik,nihw->nohw", fw[:, :, k : k + 1].astype(np.float64), sh)
    rel = np.linalg.norm(out - ref) / np.linalg.norm(ref)
    mx = np.abs(out - ref).max() / np.abs(ref).max()
    print(f"self-test H={H}: rel l2 err {rel:.3e}, max err {mx:.3e}")


# revision 23
# speedup vs baseline: 1.0494x; 1.0494x over previous
"""Trainium2 Bass kernel for nn_DiagLRConv (diag-embedded 5x5 conv, pad=2).

Math: out[n,o,h,w] = sum_{i,k} filter_w[o,i,k] * x[n,i,h+k-2,w+k-2]
(a diag_embed'ed 5x5 kernel is 5 diagonal shifts mixed through 16x16 channel
matrices).

Mapping (per NeuronCore, 2 images each, 8 cores data-parallel over batch):
  - x cast to fp16 and zero-padded on host into a flat [2,16,(H+5)*517]
    layout (517 = 2 + 512 + 3 pad columns).  fp16 rounding of x/w is the
    only approximation (~3e-4 rel l2, threshold 2e-2).
  - x is loaded ONCE (no shifted duplicate reads): each 128-row slab is
    4 row-bands of 32 output rows; band i occupies partitions 32i..32i+32
    holding [img0 16ch; img1 16ch] x 37 padded rows x 517 cols, loaded as
    one flat contiguous 38 KB/partition DMA run per (band, image).
  - Diagonal tap k of output row t reads the flat buffer at offset
    (row_in_buf)*517 + k -- no pre-shifted copies needed.
  - Matmul: 16 concurrent 32x32 tiles via tile_position=(32i,32j):
    row-band i = x data band, col-band j = output row t=4s+j.  Stationary
    [K=32,N=32] is block-diagonal: cols 0:16 = img0 out channels, cols
    16:32 = img1, so each tile computes both images at once.  5 tap-rounds
    accumulate into PSUM bank i (4 banks/step, 8 banks double-buffered);
    concurrent tiles on one column strip always target different banks.
  - PSUM -> SBUF evacuation with fp32->fp16 cast as one 2-bank (1024-col)
    copy per engine per step -- ScalarE takes bands 0,1, VectorE bands
    2,3 -- amortizing the TRN2 SBUF-op inter-instruction bubble; one
    512 KB output DMA per step on the GpSimd SWDGE path in a
    kernel-native layout; host reassembles.
"""

import numpy as np

F16 = np.float16

_COMPILED = {}

ROWS_PER_BAND = 32            # output rows per row-band per slab
BANDS = 4
SLAB = ROWS_PER_BAND * BANDS  # 128 output rows per slab
RB = ROWS_PER_BAND + 5        # 37 buffer rows per band
WPAD = 517                    # padded row length (2 + 512 + 3)
L = RB * WPAD                 # flat fp16 elems per partition per slab
STEPS = ROWS_PER_BAND // 4    # 8 steps per slab (4 rows per step per band)


def _trace_nc(H):
    import concourse.mybir as mybir
    import concourse.tile as tile
    from concourse import bacc

    F32 = mybir.dt.float32
    FP16 = mybir.dt.float16

    assert H % SLAB == 0
    G = H // SLAB

    nc = bacc.Bacc(None, target_bir_lowering=False, debug=False)
    # banded input layout, host-materialized: xp[g, 32i+16m+c, r*517+w] =
    # xpad[m, c, 128g+32i+r, w] -- so each input DMA spans all 128
    # partitions with ~5 KB/partition descriptors (DMA engines pipeline
    # across partitions only at descriptor granularity).
    xp = nc.declare_dram_parameter("xp", [G, 128, L], FP16, isOutput=False)
    wd = nc.declare_dram_parameter("wd", [128, 5, 32], FP16, isOutput=False)
    # kernel-native output layout; host reassembles:
    # y[g, s, 32j+16m+o, i, w] = out[m, o, 128g+32i+4s+j, w]
    y = nc.declare_dram_parameter("y", [G, STEPS, 128, 4, 512], FP16, isOutput=True)

    with tile.TileContext(nc) as tc:
        with (
            tc.tile_pool(name="const", bufs=1) as const,
            tc.tile_pool(name="xpool", bufs=3) as xpool,
            tc.tile_pool(name="psum", bufs=4, space="PSUM") as psum,
            tc.tile_pool(name="stpool", bufs=4) as stpool,
        ):
            wt = const.tile([128, 5, 32], FP16)
            nc.sync.dma_start(out=wt[:], in_=wd[:])

            CHUNK = 3 * WPAD  # ~3.1 KB/partition per descriptor; finer
            # chunks shrink the just-in-time arrival stalls when compute
            # catches up to the contended input stream mid-slab
            for g in range(G):
                xq = xpool.tile([128, L], FP16, tag="xq", name=f"xq{g}")
                for c0 in range(0, L, CHUNK):
                    c1 = min(L, c0 + CHUNK)
                    nc.sync.dma_start(out=xq[:, c0:c1], in_=xp[g, :, c0:c1])
                for s in range(STEPS):
                    # two 2-bank PSUM tiles per step: band i -> pair i//2,
                    # bank i%2; evac is one 1024-col copy per engine
                    pss = [
                        psum.tile([128, 2, 512], F32, tag="ps", name=f"ps{g}_{s}_{p}")
                        for p in range(2)
                    ]
                    st = stpool.tile([128, 4, 512], FP16, tag="st", name=f"st{g}_{s}")
                    # pair-phased emission: pair 0 (bands 0,1) completes and
                    # starts its ScalarE evac before pair 1 is issued, freeing
                    # its PSUM banks ~1.6 us earlier for the step-s+2 reuse
                    for p in range(2):
                        for k in range(5):
                            for i in (2 * p, 2 * p + 1):
                                for j in range(4):
                                    off = (4 * s + j + k) * WPAD + k
                                    nc.tensor.matmul(
                                        pss[p][32 * j : 32 * j + 32, i % 2, :],
                                        wt[32 * i : 32 * i + 32, k, :],
                                        xq[32 * i : 32 * i + 32, off : off + 512],
                                        start=(k == 0),
                                        stop=(k == 4),
                                        tile_position=(32 * i, 32 * j),
                                        skip_group_check=True,
                                    )
                        if p == 0:
                            nc.scalar.copy(st[:, 0:2, :], pss[0][:])
                    nc.vector.tensor_copy(st[:, 2:4, :], pss[1][:])
                    # SWDGE (gpsimd) output path: separate descriptor
                    # queues from the sync-ring input stream, so output
                    # transfers interleave with input at packet granularity
                    nc.gpsimd.dma_start(out=y[g, s], in_=st[:])
    nc.compile()
    return nc


def _get_nc(H, **kw):
    key = (H, tuple(sorted(kw.items())))
    if key not in _COMPILED:
        _COMPILED[key] = _trace_nc(H, **kw)
    return _COMPILED[key]


def _prep_inputs(x, filter_w, H):
    """x: [N,16,H,512] fp32, filter_w: [16,16,5] fp32 -> per-core in_maps."""
    N = x.shape[0]
    n_cores = N // 2
    x16 = x.astype(F16)

    wT = np.transpose(filter_w.astype(F16), (1, 2, 0))  # [i, k, o]
    wd = np.zeros((128, 5, 32), dtype=F16)
    for b in range(BANDS):
        wd[32 * b : 32 * b + 16, :, 0:16] = wT
        wd[32 * b + 16 : 32 * b + 32, :, 16:32] = wT

    G = H // SLAB
    row_starts = (
        np.arange(G)[:, None] * SLAB + np.arange(BANDS)[None, :] * ROWS_PER_BAND
    )  # [G, BANDS]
    in_maps = []
    for cid in range(n_cores):
        xpf = np.zeros((2, 16, H + 5, WPAD), dtype=F16)
        xpf[:, :, 2 : H + 2, 2:514] = x16[2 * cid : 2 * cid + 2]
        # banded layout [G, 128, L]: partition 32i+16m+c holds band i's
        # RB padded rows (with halo duplicated across bands)
        xb = xpf[:, :, row_starts[:, :, None] + np.arange(RB)]  # [2,16,G,4,RB,517]
        xb = np.transpose(xb, (2, 3, 0, 1, 4, 5)).reshape(G, 128, L)
        in_maps.append({"xp": np.ascontiguousarray(xb), "wd": wd})
    return in_maps


def _reassemble(yk, H):
    # yk [G, STEPS, 128, 4, 512]; p = 32j + 16m + o; row = 128g + 32i + 4s + j
    G = H // SLAB
    z = yk.reshape(G, STEPS, 4, 2, 16, 4, 512)      # g, s, j, m, o, i, w
    z = np.transpose(z, (3, 4, 0, 5, 1, 2, 6))      # m, o, g, i, s, j, w
    return z.reshape(2, 16, H, 512).astype(np.float32)


def kernel(x, filter_w):
    from concourse.bass_utils import run_bass_kernel_spmd

    x = np.asarray(x)
    filter_w = np.asarray(filter_w)
    N, C, H, W = x.shape
    assert (C, W) == (16, 512) and N % 2 == 0

    nc = _get_nc(H)
    in_maps = _prep_inputs(x, filter_w, H)
    n_cores = len(in_maps)
    res = run_bass_kernel_spmd(nc, in_maps, list(range(n_cores)))
    out = np.empty((N, 16, H, 512), dtype=np.float32)
    for cid in range(n_cores):
        out[2 * cid : 2 * cid + 2] = _reassemble(res.results[cid]["y"], H)
    return out


if __name__ == "__main__":
    import sys

    H = int(sys.argv[1]) if len(sys.argv) > 1 else 128
    rng = np.random.default_rng(0)
    x = rng.standard_normal((16, 16, H, 512)).astype(np.float32)
    fw = (rng.standard_normal((16, 16, 5)) * 0.1).astype(np.float32)
    out = kernel(x, fw)

    xpad = np.zeros((16, 16, H + 4, 516), dtype=np.float64)
    xpad[:, :, 2 : H + 2, 2:514] = x
    ref = np.zeros_like(out, dtype=np.float64)
    for k in range(5):
        sh = xpad[:, :, k : k + H, k : k + 512]
        ref += np.einsum("oik,nihw->nohw", fw[:, :, k : k + 1].astype(np.float64), sh)
    rel = np.linalg.norm(out - ref) / np.linalg.norm(ref)
    mx = np.abs(out - ref).max() / np.abs(ref).max()
    print(f"self-test H={H}: rel l2 err {rel:.3e}, max err {mx:.3e}")
